# revision 1
# baseline (speedup 1.0000x reference)
"""Trainium2 Bass kernel for quantized BasicBlock (DoReFa conv-bn-quant x2 + skip).

Strategy (176.4us baseline -> 76.4us):
- Data-parallel over batch: 128 images -> 16 per core across 8 cores.
- Weights are DoReFa-quantized to odd ints in [-15,15] on the HOST (numpy,
  exact fp32 replication of the reference math); the device DMAs transposed
  tap-major layouts and rounds them to f32r / fp8e4 (ints exact in both).
- conv1: 7 shifted float32r matmuls (1 cyc/row at N=512, ~2^-13 input
  mantissa rounding) over a zero-padded f32r image holding 15*x.
- conv2: stage-1 activations are exact 4-bit ints in fp8e4 -> 7 taps packed
  into 4 DoubleRow fp8 matmuls (0.5 cyc/row, K=256/instr) via custom 4-dim
  pair-window APs (constant +1-row delta; the odd tap pairs a zero-weight
  slot). Steady-state PE: (14 f32r + 8 DR) matmuls = 9216 cyc = 3.84us/img.
- identity shortcut: hh = 15x + g tensor_tensor adds split Pool/DVE
  ("split"); the last image uses a diag(225/inv2) f32r matmul into psum
  ("pe") to shorten the drain tail.
- stage1 = round(min(relu(inv1/15*ps + b1s), 15)) -> fp8 ints;
  stage2 = round(min(max(sc2*ps + b2s + 15x, 0), 15)); +2^23 magic = RNE,
  matching jnp.round.
- y stored as fp8e4 ints 0..15, decoded /15 on host (4x less out-DMA).
- schedule: distance-2 software pipeline (conv2 trails conv1 by 2 images) so
  stage1 hides behind a full image of PE work; 5 rotating padded buffers;
  8 zero-matmul warm-up ramps the PE p-state to 2.4GHz inside the DMA fill
  shadow; x(0..2) DMAs interleave with weight loads on separate queues.
"""
import numpy as np

import concourse.bass as bass
import concourse.tile as tile
from concourse import bacc, mybir, masks
from concourse.ap import AP
from concourse.bass_utils import run_bass_kernel_spmd

AF = mybir.ActivationFunctionType
OP = mybir.AluOpType
F32 = mybir.dt.float32
F32R = mybir.dt.float32r
FP8 = mybir.dt.float8e4
U8 = mybir.dt.uint8
BF16 = mybir.dt.bfloat16
MM = mybir.MatmulPerfMode.DoubleRow

B, C, H, W = 128, 128, 32, 32
NCORES = 8
BL = B // NCORES          # images per core
HP, WP = H + 2, W + 2     # zero-padded image
MAGIC = float(2 ** 23)
EPS = 1e-5
NB = 5                    # padded-buffer pipeline depth
SKIP_MODE = "split"       # "pe" | "split"
WARMUP = 6                # dummy PE warm-up matmuls before the main loop
PSB1, PSB2 = 4, 2         # psum pool depths (total banks <= 8)
LASTHALF = True           # per-half epilogue for the last image
BORDERS = True            # memset only xp borders (False: whole tile)
W1ACT = False             # w1T f32r copy on ACT (False: DVE)
HALF1 = True              # per-half ps1 tiles + per-half stage1
LASTPE = True             # last image uses the PE skip (shorter tail chain)
WZMEMSET = True           # zero the warm-up tile (off: garbage matmuls)
LASTQ = False             # quarter-granular epilogue for the last image
YLASTQUEUE = False        # last image's y DMAs on the scalar queue
PADDVE_N = 0              # first N pad-copies on DVE (fill acceleration)
IPB, SPB = 3, 3           # img / stage pool depths
OPB = 3                   # out pool depth
DIST = 2                  # conv2 trails conv1 by DIST images

TAPS = [(0, 1), (0, 2), (1, 0), (1, 1), (1, 2), (2, 0), (2, 1)]  # (0,0),(2,2) pruned
# conv2 DoubleRow slot order: pairs with constant +1-row (=WP elements) delta.
SLOT_TAPS = [(0, 1), (1, 1), (0, 2), (1, 2), (1, 0), (2, 0), (2, 1), None]


def _emit(tc, dr, bl):
    nc = tc.nc
    with tc.tile_pool(name="const", bufs=1) as cpool, \
         tc.tile_pool(name="img", bufs=IPB) as ipool, \
         tc.tile_pool(name="stage", bufs=SPB) as spool, \
         tc.tile_pool(name="out", bufs=OPB) as opool, \
         tc.tile_pool(name="ps1", bufs=PSB1, space="PSUM") as pp1, \
         tc.tile_pool(name="ps2", bufs=PSB2, space="PSUM") as pp2:

        # padded image buffers; a1 has one extra row so the k=3 DR pad-pair
        # window stays in bounds. Borders zeroed once: a1 via DMA from a host
        # zeros tensor (engine-free), xp via engine memsets.
        xp_t = [cpool.tile([C, HP, WP], F32R, tag=f"xp{k}", name=f"xp{k}")
                for k in range(NB)]
        a1_t = [cpool.tile([C, HP + 1, WP], FP8, tag=f"a1{k}", name=f"a1{k}")
                for k in range(NB)]

        # The shared DMA engine pool serializes transfers, so issue order is
        # schedule order: interleave x(0..2) with the weight loads so the
        # first three images and the weights all land as early as possible.
        def _load_x(i):
            t = ipool.tile([C, H, W], F32, tag="xsb")
            nc.sync.dma_start(t[:], dr["x"][i])
            return t

        # image 0 arrives in two row-chunks with the (bf16, small) w1t
        # transfer between them: conv1(0) h0 needs only x rows 0..16, so its
        # pad-copy fires before the rest of the image has landed
        xsb0 = ipool.tile([C, H, W], F32, tag="xsb")
        nc.sync.dma_start(xsb0[:, 0:17, :], dr["x"][0][:, 0:17, :])
        xsbs = {0: xsb0}
        # warm-up source tile; contents are never read back (zero-matmul into
        # a scratch psum bank), so it intentionally stays uninitialized when
        # WZMEMSET is off and the warm-up can start at t~0.
        wz = cpool.tile([C, 20, 32], F32R, tag="wz")
        if WZMEMSET:
            nc.vector.memset(wz[:].bitcast(F32), 0.0)

        def _zero_borders(eng, xp):
            # only the pad frame needs zeroing; pad-copy rewrites the interior
            t = xp[:].bitcast(F32)
            if not BORDERS:
                eng.memset(t, 0.0)
                return
            eng.memset(t[:, 0, :], 0.0)
            eng.memset(t[:, HP - 1, :], 0.0)
            eng.memset(t[:, 1:HP - 1, 0], 0.0)
            eng.memset(t[:, 1:HP - 1, WP - 1], 0.0)

        _zero_borders(nc.vector, xp_t[0])
        _zero_borders(nc.vector, xp_t[1])
        # xp2/xp3 border-memsets are deferred into loop iters 0/1 (Pool has
        # slack there) so they don't delay the first pad-copy

        # integer weights are exact in bf16/fp8: DMA in the small dtypes
        # (half/quarter transfer time); only w1T needs the f32r rounding copy
        w1s = cpool.tile([C, 7, C], BF16, tag="w1s")
        nc.scalar.dma_start(w1s[:], dr["w1t"])
        w1T = cpool.tile([C, 7, C], F32R, tag="w1T", name="w1T")
        if W1ACT:
            nc.scalar.activation(w1T[:], w1s[:], AF.Copy)
        else:
            nc.vector.tensor_copy(w1T[:], w1s[:])
        nc.sync.dma_start(xsb0[:, 17:H, :], dr["x"][0][:, 17:H, :])
        nc.gpsimd.tensor_scalar(xp_t[0][:, 1:18, 1:W + 1], xsb0[:, 0:17, :],
                                15.0, None, OP.mult)
        nc.gpsimd.tensor_scalar(xp_t[0][:, 18:H + 1, 1:W + 1],
                                xsb0[:, 17:H, :], 15.0, None, OP.mult)
        w2T8 = cpool.tile([C, 8, C], FP8, tag="w2T8", name="w2T8")
        nc.scalar.dma_start(w2T8[:], dr["w2t8"])
        xsbs[1] = _load_x(1)
        xsbs[2] = _load_x(2)
        # bn affines, host-folded: [inv1/15, b1s, sc2, b2s, 225/inv2]
        bnp = cpool.tile([C, 5], F32, tag="bnp")
        nc.scalar.dma_start(bnp[:], dr["bnp"])
        inv1, b1s, sc2, b2s, dcol = (bnp[:, k:k + 1] for k in range(5))

        # a1 borders land via DMA, after the latency-critical transfers
        for k in range(NB):
            nc.scalar.dma_start(a1_t[k][:].bitcast(U8), dr["z8"])

        if SKIP_MODE == "pe" or LASTPE:
            ident = cpool.tile([C, C], F32, tag="ident")
            masks.make_identity(nc, ident[:])
            # diag(225/inv2) / 15 (xp holds 15x): sc2 * Dg @ (15x) == 15x
            dc15 = cpool.tile([C, 1], F32, tag="dc15")
            nc.vector.tensor_scalar_mul(dc15[:], dcol, 1.0 / 15.0)
            Dg = cpool.tile([C, C], F32R, tag="Dg", name="Dg")
            nc.scalar.activation(Dg[:], ident[:], AF.Copy, scale=dc15[:])

        if WARMUP:
            # ramp the PE p-state on zero matmuls (gated only on the tiny wz
            # memset) so the first real conv1 starts at full clock
            wps = pp1.tile([C, 512] if HALF1 else [C, 1024], F32, tag="ps")
            for _ in range(WARMUP):
                nc.tensor.matmul(wps[:, 0:512], wz[:, 0:4, :], wz[:, 4:20, :],
                                 start=True, stop=True)

        def _conv1(i):
            xp = xp_t[i % NB]
            xsb = xsbs.pop(i)
            # pad-copy: xp = f32r(15*x); the 1/15 is host-folded into inv1 and
            # the skip path uses xp directly. (image 0's pads were pre-emitted)
            if i > 0:
                eng = nc.vector if i < PADDVE_N else nc.gpsimd
                eng.tensor_scalar(xp[:, 1:H + 1, 1:W + 1], xsb[:],
                                  15.0, None, OP.mult)

            a1 = a1_t[i % NB]
            if HALF1:
                # per-half ps1 tiles: halves the psum WAR window on rotation
                for h in (0, 1):
                    rs = 16 * h
                    ps1 = pp1.tile([C, 512], F32, tag="ps")
                    for t, (ky, kx) in enumerate(TAPS):
                        nc.tensor.matmul(ps1[:], w1T[:, t, :],
                                         xp[:, rs + ky:rs + ky + 16, kx:kx + W],
                                         start=(t == 0), stop=(t == 6))
                    rt = spool.tile([C, H, W], F32, tag="st_r", name="rt")
                    r = rt[:, rs:rs + 16, :]
                    nc.scalar.activation(
                        r, ps1[:].rearrange("c (h w) -> c h w", h=16),
                        AF.Relu, bias=b1s, scale=inv1)
                    qt = spool.tile([C, H, W], F32, tag="st_q", name="qt")
                    q = qt[:, rs:rs + 16, :]
                    nc.vector.tensor_scalar(q, r, 15.0, MAGIC, OP.min, OP.add)
                    nc.vector.tensor_scalar(a1[:, 1 + rs:17 + rs, 1:W + 1], q,
                                            MAGIC, None, OP.subtract)
                return
            ps1 = pp1.tile([C, 1024], F32, tag="ps")
            for h in (0, 1):
                rs = 16 * h
                out_ap = ps1[:, 512 * h:512 * (h + 1)]
                for t, (ky, kx) in enumerate(TAPS):
                    nc.tensor.matmul(out_ap, w1T[:, t, :],
                                     xp[:, rs + ky:rs + ky + 16, kx:kx + W],
                                     start=(t == 0), stop=(t == 6))
            # stage1: a1 = round(min(relu(inv1*ps + b1s), 15)), ints in fp8
            r = spool.tile([C, H, W], F32, tag="st_r")
            nc.scalar.activation(r[:], ps1[:].rearrange("c (h w) -> c h w", h=H),
                                 AF.Relu, bias=b1s, scale=inv1)
            q = spool.tile([C, H, W], F32, tag="st_q")
            nc.vector.tensor_scalar(q[:], r[:], 15.0, MAGIC, OP.min, OP.add)
            nc.vector.tensor_scalar(a1[:, 1:H + 1, 1:W + 1], q[:],
                                    MAGIC, None, OP.subtract)

        def _conv2(i, per_half=False, skip=None):
            skip = skip or SKIP_MODE
            xp = xp_t[i % NB]
            a1 = a1_t[i % NB]
            y8 = opool.tile([C, H, W], FP8, tag="y8")
            full = a1[:]
            pstride = full.ap[0][0]
            if per_half:
                # separate per-half psum tiles from pp1 (4 bufs, already
                # evacuated an image earlier) -> no WAR against g(13)/g(14)
                ps2h = [pp1.tile([C, 512], F32, tag="ps", name="ps2h")
                        for _ in (0, 1)]
                psv = lambda h: ps2h[h][:]
                psq = lambda rs, re: ps2h[rs // 16][:, (rs % 16) * W:((re - 1) % 16 + 1) * W]
            else:
                ps2 = pp2.tile([C, 1024], F32, tag="ps")
                psv = lambda h: ps2[:, 512 * h:512 * (h + 1)]
                psq = lambda rs, re: ps2[:, rs * W:re * W]

            def _mm(h):
                rs = 16 * h
                out_ap = psv(h)
                if skip == "pe":
                    nc.tensor.matmul(out_ap, Dg[:],
                                     xp[:, rs + 1:rs + 17, 1:W + 1],
                                     start=True, stop=False)
                for k in range(4):
                    ky, kx = SLOT_TAPS[2 * k]
                    off = (rs + ky) * WP + kx
                    rhs = AP(full.tensor, full.offset + off,
                             [[pstride, C], [WP, 2], [WP, 16], [1, W]])
                    nc.tensor.matmul(out_ap, w2T8[:, 2 * k:2 * k + 2, :], rhs,
                                     start=(skip != "pe" and k == 0),
                                     stop=(k == 3), perf_mode=MM)

            def _st2(h0, h1, last=False):
                # rows [16*h0, 16*h1): y8 = round(min(max(sc2*ps+b2s+15x,0),15))
                rs, re = int(16 * h0), int(16 * h1)
                ps_ap = psq(rs, re).rearrange("c (h w) -> c h w", h=re - rs)
                gt = spool.tile([C, H, W], F32, tag="st_g", name="gt")
                g = gt[:, rs:re, :]
                if skip == "pe":
                    nc.scalar.activation(g, ps_ap, AF.Relu, bias=b2s, scale=sc2)
                    pt = spool.tile([C, H, W], F32, tag="st_p", name="pt")
                    p = pt[:, rs:re, :]
                    nc.vector.tensor_scalar(p, g, 15.0, MAGIC, OP.min, OP.add)
                    nc.vector.tensor_scalar(y8[:, rs:re, :], p,
                                            MAGIC, None, OP.subtract)
                else:
                    nc.scalar.activation(g, ps_ap, AF.Identity, bias=b2s,
                                         scale=sc2)
                    hht = spool.tile([C, H, W], F32, tag="st_h", name="hht")
                    hh = hht[:, rs:re, :]
                    xpi = xp[:, 1 + rs:1 + re, 1:W + 1].bitcast(F32)
                    mid = (rs + re) // 2
                    nc.gpsimd.tensor_tensor(hh[:, :mid - rs, :],
                                            xpi[:, :mid - rs, :],
                                            g[:, :mid - rs, :], OP.add)
                    nc.vector.tensor_tensor(hh[:, mid - rs:, :],
                                            xpi[:, mid - rs:, :],
                                            g[:, mid - rs:, :], OP.add)
                    pt = spool.tile([C, H, W], F32, tag="st_p", name="pt")
                    p = pt[:, rs:re, :]
                    nc.vector.tensor_scalar(p, hh, 0.0, MAGIC, OP.max, OP.add)
                    nc.vector.tensor_scalar(y8[:, rs:re, :], p, MAGIC, 15.0,
                                            OP.subtract, OP.min)
                eng = nc.scalar if (per_half and YLASTQUEUE) else nc.sync
                eng.dma_start(dr["y"][i][:, rs:re, :], y8[:, rs:re, :])

            if per_half and LASTQ:
                # taper: full first half, then 12+4 rows so the final serial
                # epilogue chain covers only 4 rows
                _mm(0)
                _st2(0, 1)
                _mm(1)
                _st2(1, 1.75)
                _st2(1.75, 2)
            elif per_half:
                _mm(0)
                _st2(0, 1)
                _mm(1)
                _st2(1, 2)
            else:
                _mm(0)
                _mm(1)
                _st2(0, 2)

        # distance-2 software pipeline: conv2(i) trails conv1(i) by two
        # iterations, so stage1(i) has a whole extra image of PE work
        # (conv1(i+1), conv1(i+2)) to hide behind.
        for i in range(bl + DIST):
            if i < bl:
                _conv1(i)
                if i + 2 < NB:
                    # deferred xp border memsets; Pool has slack here
                    _zero_borders(nc.gpsimd, xp_t[i + 2])
                nxt = i + IPB
                if 2 < nxt < bl and nxt not in xsbs:
                    xsbs[nxt] = _load_x(nxt)
                # with IPB>3 the pre-loop only loaded 0..2; catch up here
                for j in range(3, min(i + IPB, bl)):
                    if j not in xsbs and j > i:
                        xsbs[j] = _load_x(j)
            if i >= DIST:
                _conv2(i - DIST, per_half=(LASTHALF and i == bl + DIST - 1),
                       skip="pe" if (LASTPE and i == bl + DIST - 1) else None)


def _build(bl=BL):
    nc = bacc.Bacc("TRN2", target_bir_lowering=False, debug=False,
                   enable_asserts=False, num_devices=NCORES)
    dr = {}
    dr["x"] = nc.dram_tensor("x", [bl, C, H, W], F32, kind="ExternalInput").ap()
    dr["w1t"] = nc.dram_tensor("w1t", [C, 7, C], BF16, kind="ExternalInput").ap()
    dr["w2t8"] = nc.dram_tensor("w2t8", [C, 8, C], FP8, kind="ExternalInput").ap()
    dr["bnp"] = nc.dram_tensor("bnp", [C, 5], F32, kind="ExternalInput").ap()
    dr["z8"] = nc.dram_tensor("z8", [C, (HP + 1) * WP], U8, kind="ExternalInput").ap()
    dr["y"] = nc.dram_tensor("y", [bl, C, H, W], FP8, kind="ExternalOutput").ap()
    with tile.TileContext(nc) as tc:
        _emit(tc, dr, bl)
    nc.compile()
    return nc


_CACHED = None


def _host_prep(inputs):
    """Replicate the reference's fp32 weight-quant + BN folding in numpy."""
    f = lambda v: np.asarray(v, dtype=np.float32)

    def wint(w):
        t = np.tanh(f(w))
        m = np.abs(t).max()
        t2 = t / (np.float32(2.0) * m) + np.float32(0.5)
        v = t2 * np.float32(15.0)
        return (np.float32(2.0) * np.round(v) - np.float32(15.0)).astype(np.float32)

    wi1 = wint(inputs["w1"]).reshape(C, C, 3, 3)
    wi2 = wint(inputs["w2"]).reshape(C, C, 3, 3)
    w1t = np.empty((C, 7, C), np.float32)
    for t, (ky, kx) in enumerate(TAPS):
        w1t[:, t, :] = wi1[:, :, ky, kx].T
    w2t8 = np.zeros((C, 8, C), np.float32)
    for s, st in enumerate(SLOT_TAPS):
        if st is not None:
            w2t8[:, s, :] = wi2[:, :, st[0], st[1]].T

    g1, b1, m1, v1, g2, b2, m2, v2 = (
        f(inputs[k]) for k in ("gamma1", "beta1", "mean1", "var1",
                               "gamma2", "beta2", "mean2", "var2"))
    inv1 = g1 / np.sqrt(v1 + np.float32(EPS))
    inv2 = g2 / np.sqrt(v2 + np.float32(EPS))
    b1s = np.float32(15.0) * (b1 - m1 * inv1)
    b2s = np.float32(15.0) * (b2 - m2 * inv2)
    sc2 = inv2 / np.float32(15.0)
    # the padded image holds 15*x: fold 1/15 into inv1; dcol for the PE skip
    inv1_15 = inv1 / np.float32(15.0)
    dcol = np.float32(225.0) / inv2
    bnp = np.ascontiguousarray(np.stack([inv1_15, b1s, sc2, b2s, dcol], axis=1))
    import ml_dtypes
    return (np.ascontiguousarray(w1t.astype(ml_dtypes.bfloat16)),
            np.ascontiguousarray(w2t8.astype(ml_dtypes.float8_e4m3fn)), bnp)


def _in_maps(inputs, bl=BL, ncores=NCORES):
    x = np.ascontiguousarray(np.asarray(inputs["x"], dtype=np.float32))
    w1t, w2t8, bnp = _host_prep(inputs)
    base = {"w1t": w1t, "w2t8": w2t8, "bnp": bnp,
            "z8": np.zeros((C, (HP + 1) * WP), np.uint8)}
    maps = []
    for c in range(ncores):
        m = dict(base)
        m["x"] = np.ascontiguousarray(x[c * bl:(c + 1) * bl])
        maps.append(m)
    return maps


def _run(inputs, trace=False):
    global _CACHED
    if _CACHED is None:
        _CACHED = _build()
    res = run_bass_kernel_spmd(_CACHED, _in_maps(inputs),
                               core_ids=list(range(NCORES)), trace=trace)
    y = np.concatenate([np.asarray(res.results[c]["y"]).astype(np.float32)
                        for c in range(NCORES)], axis=0)
    return y * np.float32(1.0 / 15.0), res


def kernel(**inputs) -> np.ndarray:
    y, _ = _run(inputs, trace=False)
    return y



# revision 6
# speedup vs baseline: 1.0500x; 1.0500x over previous
"""Trainium2 Bass kernel for quantized BasicBlock (DoReFa conv-bn-quant x2 + skip).

Strategy (75.7us prior -> target ~42us):
- Data-parallel over batch: 128 images -> 16 per core across 8 cores.
- Weights DoReFa-quantized to odd ints in [-15,15] on the HOST (exact fp32
  replication of the reference math); exact in fp8e4.
- x is split on the host: hi = fp8(15x), lo = fp8(15x - hi); sent as a
  padded row-interleaved [C, 34, 2, 34] fp8 tensor. conv1 = 7 fp8 DoubleRow
  matmuls per half (pair = (hi,lo) windows of the same tap, both slots
  carrying the same integer weight) -> K=256, 0.5 cyc/row.
- stage1 uses the HW's round-to-nearest-even f32->uint8 convert: one ACT
  affine (Identity, scale=inv1/15, bias=15(b-mu*inv)) + one DVE dual-op
  tensor_scalar (max 0, min 15) writing uint8 -> a1 holds exact ints 0..15.
- conv2 reads a1 BITCAST as fp8e4: uint8 k in 0..15 bitcasts to the exactly
  linear subnormal/low-normal values k*2^-9, so psum = 2^-9 * int-conv;
  the 2^9 is folded into the stage-2 scale. 4 DR matmuls per half.
- skip: host also sends xf16 = fp16(15x); hh = g + xf16 (one mixed-dtype
  tensor_tensor on Pool), y = RNE-uint8(clip(hh,0,15)) on DVE, decoded /15
  on host.
- schedule: distance-2 software pipeline; 5 rotating input buffers; PE
  warm-up matmuls ramp the p-state during the DMA fill; last image runs
  per-half to shorten the drain tail.
Steady state per image: PE 22 DR matmuls (2395ns), DVE 2394ns, ACT 2190ns,
Pool 2187ns, DMA 2076ns.
"""
import numpy as np

import concourse.bass as bass
import concourse.tile as tile
from concourse import bacc, mybir, masks
from concourse.ap import AP
from concourse.bass_utils import run_bass_kernel_spmd

AF = mybir.ActivationFunctionType
OP = mybir.AluOpType
F32 = mybir.dt.float32
F32R = mybir.dt.float32r
FP8 = mybir.dt.float8e4
F16 = mybir.dt.float16
U8 = mybir.dt.uint8
MM = mybir.MatmulPerfMode.DoubleRow

B, C, H, W = 128, 128, 32, 32
NCORES = 8
BL = B // NCORES          # images per core
HP, WP = H + 2, W + 2     # zero-padded image
EPS = 1e-5
NB = 5                    # rotating input-buffer depth
WARMUP = 6                # dummy PE warm-up matmuls before the main loop
PSB1, PSB2 = 2, 2         # psum pool depths ([C,1024] tiles; 2 banks each)
SPB = 3                   # stage pool depth
OPB = 3                   # out pool depth
DIST = 2                  # conv2 trails conv1 by DIST images
PREF = 3                  # input prefetch distance (images ahead)
LASTHALF = True           # per-half epilogue for the last image
HH_ENG = "pool"           # engine for hh = g + xf16
A1U_ENG = "dve"           # engine for the stage1 uint8 quantize
Y8_ENG = "dve"            # engine for the stage2 uint8 quantize
DEBUG_A1 = False          # add a debug output dumping stage-1 a1 ints

TAPS = [(0, 1), (0, 2), (1, 0), (1, 1), (1, 2), (2, 0), (2, 1)]  # (0,0),(2,2) pruned
# conv2 DoubleRow slot order: pairs with constant +1-row (=WP elements) delta.
SLOT_TAPS = [(0, 1), (1, 1), (0, 2), (1, 2), (1, 0), (2, 0), (2, 1), None]


def _emit(tc, dr, bl):
    nc = tc.nc
    with tc.tile_pool(name="const", bufs=1) as cpool, \
         tc.tile_pool(name="stage", bufs=SPB) as spool, \
         tc.tile_pool(name="out", bufs=OPB) as opool, \
         tc.tile_pool(name="ps1", bufs=PSB1, space="PSUM") as pp1, \
         tc.tile_pool(name="ps2", bufs=PSB2, space="PSUM") as pp2:

        # rotating input buffers: xhl holds the padded (hi,lo) fp8 planes
        # (borders pre-zeroed on the host), a1 gets zero borders via DMA.
        xhl_t = [cpool.tile([C, HP, 2, WP], FP8, tag=f"xhl{k}", name=f"xhl{k}")
                 for k in range(NB)]
        a1_t = [cpool.tile([C, HP + 1, WP], U8, tag=f"a1{k}", name=f"a1{k}")
                for k in range(NB)]
        xf_t = [cpool.tile([C, H, W], F16, tag=f"xf{k}", name=f"xf{k}")
                for k in range(NB)]

        # image 0 lands in two chunks with the weights between them: conv1(0)
        # h0 only needs padded rows 0..17.
        nc.sync.dma_start(xhl_t[0][:, 0:18, :, :], dr["xhl"][0][:, 0:18, :, :])
        w1t8 = cpool.tile([C, 14, C], FP8, tag="w1t8", name="w1t8")
        nc.scalar.dma_start(w1t8[:], dr["w1t8"])
        nc.sync.dma_start(xhl_t[0][:, 18:HP, :, :], dr["xhl"][0][:, 18:HP, :, :])
        w2t8 = cpool.tile([C, 8, C], FP8, tag="w2t8", name="w2t8")
        nc.scalar.dma_start(w2t8[:], dr["w2t8"])
        # bn affines, host-folded: [inv1/15, b1s, 512*inv2/15, b2s]
        bnp = cpool.tile([C, 4], F32, tag="bnp")
        nc.scalar.dma_start(bnp[:], dr["bnp"])
        inv1, b1s, sc2, b2s = (bnp[:, k:k + 1] for k in range(4))

        nc.sync.dma_start(xhl_t[1][:], dr["xhl"][1])
        nc.scalar.dma_start(xf_t[0][:], dr["xf"][0])
        nc.sync.dma_start(xhl_t[2][:], dr["xhl"][2])
        nc.scalar.dma_start(xf_t[1][:], dr["xf"][1])
        nc.scalar.dma_start(xf_t[2][:], dr["xf"][2])

        # a1 borders (and the slot-7 overrun row) land via DMA zeros
        for k in range(NB):
            nc.scalar.dma_start(a1_t[k][:], dr["z8"])

        # warm-up: ramp the PE p-state on zero matmuls so the first real
        # conv1 starts closer to full clock
        wz = cpool.tile([C, 20, 32], F32R, tag="wz")
        nc.vector.memset(wz[:].bitcast(F32), 0.0)
        if WARMUP:
            wps = pp1.tile([C, 1024], F32, tag="ps")
            for _ in range(WARMUP):
                nc.tensor.matmul(wps[:, 0:512], wz[:, 0:4, :], wz[:, 4:20, :],
                                 start=True, stop=True)

        def _dr_win(full, pstride, row, kx, nrows=16):
            # (hi,lo) pair window: [part, pair(2), rows, cols]; pair delta is
            # one plane (=WP elements)
            off = row * 2 * WP + kx
            return AP(full.tensor, full.offset + off,
                      [[pstride, C], [WP, 2], [2 * WP, nrows], [1, W]])

        def _conv1(i):
            xhl = xhl_t[i % NB]
            a1 = a1_t[i % NB]
            full = xhl[:]
            pstride = full.ap[0][0]
            ps1 = pp1.tile([C, 1024], F32, tag="ps")
            for h in (0, 1):
                rs = 16 * h
                out_ap = ps1[:, 512 * h:512 * (h + 1)]
                for t, (ky, kx) in enumerate(TAPS):
                    nc.tensor.matmul(out_ap, w1t8[:, 2 * t:2 * t + 2, :],
                                     _dr_win(full, pstride, rs + ky, kx),
                                     start=(t == 0), stop=(t == 6),
                                     perf_mode=MM)
            # stage1: a1 = rne_u8(clip(inv1/15*ps + b1s, 0, 15)) in 2 ops
            rt = spool.tile([C, H, W], F32, tag="st_r", name="rt")
            nc.scalar.activation(rt[:],
                                 ps1[:].rearrange("c (h w) -> c h w", h=H),
                                 AF.Identity, bias=b1s, scale=inv1)
            eng = nc.vector if A1U_ENG == "dve" else nc.gpsimd
            eng.tensor_scalar(a1[:, 1:H + 1, 1:W + 1], rt[:],
                              0.0, 15.0, OP.max, OP.min)
            if DEBUG_A1:
                nc.sync.dma_start(dr["a1d"][i], a1[:, 1:H + 1, 1:W + 1])

        def _conv2(i, per_half=False):
            xhl = xhl_t[i % NB]
            a1 = a1_t[i % NB]
            xf = xf_t[i % NB]
            y8 = opool.tile([C, H, W], U8, tag="y8")
            full = a1[:].bitcast(FP8)
            pstride = full.ap[0][0]
            ps2 = pp2.tile([C, 1024], F32, tag="ps")

            def _mm(h):
                rs = 16 * h
                out_ap = ps2[:, 512 * h:512 * (h + 1)]
                for k in range(4):
                    ky, kx = SLOT_TAPS[2 * k]
                    off = (rs + ky) * WP + kx
                    rhs = AP(full.tensor, full.offset + off,
                             [[pstride, C], [WP, 2], [WP, 16], [1, W]])
                    nc.tensor.matmul(out_ap, w2t8[:, 2 * k:2 * k + 2, :], rhs,
                                     start=(k == 0), stop=(k == 3),
                                     perf_mode=MM)

            def _st2(h0, h1):
                # rows [16*h0, 16*h1): y8 = rne_u8(clip(sc2*ps+b2s+15x, 0, 15))
                rs, re = 16 * h0, 16 * h1
                ps_ap = ps2[:, rs * W:re * W].rearrange(
                    "c (h w) -> c h w", h=re - rs)
                gt = spool.tile([C, H, W], F32, tag="st_g", name="gt")
                g = gt[:, rs:re, :]
                nc.scalar.activation(g, ps_ap, AF.Identity, bias=b2s,
                                     scale=sc2)
                hht = spool.tile([C, H, W], F32, tag="st_h", name="hht")
                hh = hht[:, rs:re, :]
                heng = nc.gpsimd if HH_ENG == "pool" else nc.vector
                if per_half:
                    # drain tail: run the add on both engines concurrently
                    heng = nc.gpsimd if h0 == 0 else nc.vector
                heng.tensor_tensor(hh, g, xf[:, rs:re, :], OP.add)
                yeng = nc.vector if Y8_ENG == "dve" else nc.gpsimd
                yeng.tensor_scalar(y8[:, rs:re, :], hh, 0.0, 15.0,
                                   OP.max, OP.min)
                nc.sync.dma_start(dr["y"][i][:, rs:re, :], y8[:, rs:re, :])

            if per_half:
                _mm(0)
                _st2(0, 1)
                _mm(1)
                _st2(1, 2)
            else:
                _mm(0)
                _mm(1)
                _st2(0, 2)

        # distance-2 software pipeline: conv2(i) trails conv1(i) by two
        # iterations so stage1(i) hides behind conv1(i+1)/conv1(i+2) PE work.
        for i in range(bl + DIST):
            nxt = i + PREF
            if i < bl:
                _conv1(i)
                if 2 < nxt < bl:
                    nc.sync.dma_start(xhl_t[nxt % NB][:], dr["xhl"][nxt])
            if i >= DIST:
                _conv2(i - DIST,
                       per_half=(LASTHALF and i == bl + DIST - 1))
            # xf(nxt) lands in the buffer conv2(i-DIST) just read; issue the
            # prefetch after that read so the WAR resolves correctly.
            if 2 < nxt < bl:
                nc.scalar.dma_start(xf_t[nxt % NB][:], dr["xf"][nxt])


def _build(bl=BL):
    nc = bacc.Bacc("TRN2", target_bir_lowering=False, debug=False,
                   enable_asserts=False, num_devices=NCORES)
    dr = {}
    dr["xhl"] = nc.dram_tensor("xhl", [bl, C, HP, 2, WP], FP8,
                               kind="ExternalInput").ap()
    dr["xf"] = nc.dram_tensor("xf", [bl, C, H, W], F16,
                              kind="ExternalInput").ap()
    dr["w1t8"] = nc.dram_tensor("w1t8", [C, 14, C], FP8,
                                kind="ExternalInput").ap()
    dr["w2t8"] = nc.dram_tensor("w2t8", [C, 8, C], FP8,
                                kind="ExternalInput").ap()
    dr["bnp"] = nc.dram_tensor("bnp", [C, 4], F32, kind="ExternalInput").ap()
    dr["z8"] = nc.dram_tensor("z8", [C, (HP + 1) * WP], U8,
                              kind="ExternalInput").ap()
    dr["y"] = nc.dram_tensor("y", [bl, C, H, W], U8, kind="ExternalOutput").ap()
    if DEBUG_A1:
        dr["a1d"] = nc.dram_tensor("a1d", [bl, C, H, W], U8,
                                   kind="ExternalOutput").ap()
    with tile.TileContext(nc) as tc:
        _emit(tc, dr, bl)
    nc.compile()
    return nc


_CACHED = None


def _host_prep(inputs):
    """Replicate the reference's fp32 weight-quant + BN folding in numpy."""
    import ml_dtypes
    f = lambda v: np.asarray(v, dtype=np.float32)

    def wint(w):
        t = np.tanh(f(w))
        m = np.abs(t).max()
        t2 = t / (np.float32(2.0) * m) + np.float32(0.5)
        v = t2 * np.float32(15.0)
        return (np.float32(2.0) * np.round(v) - np.float32(15.0)).astype(np.float32)

    wi1 = wint(inputs["w1"]).reshape(C, C, 3, 3)
    wi2 = wint(inputs["w2"]).reshape(C, C, 3, 3)
    w1t8 = np.empty((C, 14, C), np.float32)
    for t, (ky, kx) in enumerate(TAPS):
        w1t8[:, 2 * t, :] = wi1[:, :, ky, kx].T
        w1t8[:, 2 * t + 1, :] = wi1[:, :, ky, kx].T
    w2t8 = np.zeros((C, 8, C), np.float32)
    for s, st in enumerate(SLOT_TAPS):
        if st is not None:
            w2t8[:, s, :] = wi2[:, :, st[0], st[1]].T

    g1, b1, m1, v1, g2, b2, m2, v2 = (
        f(inputs[k]) for k in ("gamma1", "beta1", "mean1", "var1",
                               "gamma2", "beta2", "mean2", "var2"))
    inv1 = g1 / np.sqrt(v1 + np.float32(EPS))
    inv2 = g2 / np.sqrt(v2 + np.float32(EPS))
    b1s = np.float32(15.0) * (b1 - m1 * inv1)
    b2s = np.float32(15.0) * (b2 - m2 * inv2)
    inv1_15 = inv1 / np.float32(15.0)
    sc2p = np.float32(512.0) * inv2 / np.float32(15.0)
    bnp = np.ascontiguousarray(np.stack([inv1_15, b1s, sc2p, b2s], axis=1))
    return (np.ascontiguousarray(w1t8.astype(ml_dtypes.float8_e4m3fn)),
            np.ascontiguousarray(w2t8.astype(ml_dtypes.float8_e4m3fn)), bnp)


def _split_x(x):
    """Host hi/lo fp8 split of 15x into the padded interleaved layout."""
    import ml_dtypes
    xs = np.float32(15.0) * np.asarray(x, np.float32)  # [n, C, H, W]
    hi = xs.astype(ml_dtypes.float8_e4m3fn)
    lo = (xs - hi.astype(np.float32)).astype(ml_dtypes.float8_e4m3fn)
    n = xs.shape[0]
    xhl = np.zeros((n, C, HP, 2, WP), ml_dtypes.float8_e4m3fn)
    xhl[:, :, 1:H + 1, 0, 1:W + 1] = hi
    xhl[:, :, 1:H + 1, 1, 1:W + 1] = lo
    return np.ascontiguousarray(xhl), np.ascontiguousarray(xs.astype(np.float16))


def _in_maps(inputs, bl=BL, ncores=NCORES):
    w1t8, w2t8, bnp = _host_prep(inputs)
    base = {"w1t8": w1t8, "w2t8": w2t8, "bnp": bnp,
            "z8": np.zeros((C, (HP + 1) * WP), np.uint8)}
    x = np.asarray(inputs["x"], dtype=np.float32)
    maps = []
    for c in range(ncores):
        m = dict(base)
        m["xhl"], m["xf"] = _split_x(x[c * bl:(c + 1) * bl])
        maps.append(m)
    return maps


def _run(inputs, trace=False):
    global _CACHED
    if _CACHED is None:
        _CACHED = _build()
    res = run_bass_kernel_spmd(_CACHED, _in_maps(inputs),
                               core_ids=list(range(NCORES)), trace=trace)
    y = np.concatenate([np.asarray(res.results[c]["y"]).astype(np.float32)
                        for c in range(NCORES)], axis=0)
    return y * np.float32(1.0 / 15.0), res


def kernel(**inputs) -> np.ndarray:
    y, _ = _run(inputs, trace=False)
    return y


# revision 9
# speedup vs baseline: 1.1371x; 1.0830x over previous
"""Trainium2 Bass kernel for quantized BasicBlock (DoReFa conv-bn-quant x2 + skip).

Strategy (75.7us prior -> target ~42us):
- Data-parallel over batch: 128 images -> 16 per core across 8 cores.
- Weights DoReFa-quantized to odd ints in [-15,15] on the HOST (exact fp32
  replication of the reference math); exact in fp8e4.
- x is split on the host: hi = fp8(15x), lo = fp8(15x - hi); sent as a
  padded row-interleaved [C, 34, 2, 34] fp8 tensor. conv1 = 7 fp8 DoubleRow
  matmuls per half (pair = (hi,lo) windows of the same tap, both slots
  carrying the same integer weight) -> K=256, 0.5 cyc/row.
- stage1 uses the HW's round-to-nearest-even f32->uint8 convert: one ACT
  affine (Identity, scale=inv1/15, bias=15(b-mu*inv)) + one DVE dual-op
  tensor_scalar (max 0, min 15) writing uint8 -> a1 holds exact ints 0..15.
- conv2 reads a1 BITCAST as fp8e4: uint8 k in 0..15 bitcasts to the exactly
  linear subnormal/low-normal values k*2^-9, so psum = 2^-9 * int-conv;
  the 2^9 is folded into the stage-2 scale. 4 DR matmuls per half.
- skip: host also sends xf16 = fp16(15x); hh = g + xf16 (one mixed-dtype
  tensor_tensor on Pool), y = RNE-uint8(clip(hh,0,15)) on DVE, decoded /15
  on host.
- schedule: distance-2 software pipeline; 5 rotating input buffers; PE
  warm-up matmuls ramp the p-state during the DMA fill; last image runs
  per-half to shorten the drain tail.
Steady state per image: PE 22 DR matmuls (2395ns), DVE 2394ns, ACT 2190ns,
Pool 2187ns, DMA 2076ns.
"""
import numpy as np

import concourse.bass as bass
import concourse.tile as tile
from concourse import bacc, mybir, masks
from concourse.ap import AP
from concourse.bass_utils import run_bass_kernel_spmd

AF = mybir.ActivationFunctionType
OP = mybir.AluOpType
F32 = mybir.dt.float32
F32R = mybir.dt.float32r
FP8 = mybir.dt.float8e4
F16 = mybir.dt.float16
U8 = mybir.dt.uint8
MM = mybir.MatmulPerfMode.DoubleRow

B, C, H, W = 128, 128, 32, 32
NCORES = 8
BL = B // NCORES          # images per core
HP, WP = H + 2, W + 2     # zero-padded image
EPS = 1e-5
NB = 5                    # rotating input-buffer depth
WARMUP = 6                # dummy PE warm-up matmuls before the main loop
PSB1, PSB2 = 2, 2         # psum pool depths ([C,1024] tiles; 2 banks each)
SPB = 3                   # stage pool depth
OPB = 3                   # out pool depth
DIST = 2                  # conv2 trails conv1 by DIST images
PREF = 3                  # input prefetch distance (images ahead)
LASTHALF = True           # per-half epilogue for the last image
HH_ENG = "dve"            # engine for hh = g + xf16 (DVE 2x: ~594ns)
A1U_ENG = "dve"           # engine for the stage1 uint8 quantize
Y8_ENG = "dve"            # engine for the stage2 uint8 quantize
DEBUG_A1 = False          # add a debug output dumping stage-1 a1 ints

TAPS = [(0, 1), (0, 2), (1, 0), (1, 1), (1, 2), (2, 0), (2, 1)]  # (0,0),(2,2) pruned
# conv2 DoubleRow slot order: pairs with constant +1-row (=WP elements) delta.
SLOT_TAPS = [(0, 1), (1, 1), (0, 2), (1, 2), (1, 0), (2, 0), (2, 1), None]


def _emit(tc, dr, bl):
    nc = tc.nc
    with tc.tile_pool(name="const", bufs=1) as cpool, \
         tc.tile_pool(name="stage", bufs=SPB) as spool, \
         tc.tile_pool(name="out", bufs=OPB) as opool, \
         tc.tile_pool(name="ps1", bufs=PSB1, space="PSUM") as pp1, \
         tc.tile_pool(name="ps2", bufs=PSB2, space="PSUM") as pp2:

        # rotating input buffers: xhl holds the padded (hi,lo) fp8 planes
        # (borders pre-zeroed on the host), a1 gets zero borders via DMA.
        xhl_t = [cpool.tile([C, HP, 2, WP], FP8, tag=f"xhl{k}", name=f"xhl{k}")
                 for k in range(NB)]
        a1_t = [cpool.tile([C, HP + 1, WP], U8, tag=f"a1{k}", name=f"a1{k}")
                for k in range(NB)]
        xf_t = [cpool.tile([C, H, W], F16, tag=f"xf{k}", name=f"xf{k}")
                for k in range(NB)]

        # image 0 lands in two chunks with the weights between them: conv1(0)
        # h0 only needs padded rows 0..17.
        nc.sync.dma_start(xhl_t[0][:, 0:18, :, :], dr["xhl"][0][:, 0:18, :, :])
        w1t8 = cpool.tile([C, 14, C], FP8, tag="w1t8", name="w1t8")
        nc.scalar.dma_start(w1t8[:], dr["w1t8"])
        nc.sync.dma_start(xhl_t[0][:, 18:HP, :, :], dr["xhl"][0][:, 18:HP, :, :])
        w2t8 = cpool.tile([C, 8, C], FP8, tag="w2t8", name="w2t8")
        nc.scalar.dma_start(w2t8[:], dr["w2t8"])
        # bn affines, host-folded: [inv1/15, b1s, 512*inv2/15, b2s]
        bnp = cpool.tile([C, 4], F32, tag="bnp")
        nc.scalar.dma_start(bnp[:], dr["bnp"])
        inv1, b1s, sc2, b2s = (bnp[:, k:k + 1] for k in range(4))

        # a1(0)/a1(1) borders must land before conv2(0)/conv2(1); the rest of
        # the zero fills can trail the early image/skip transfers.
        nc.scalar.dma_start(a1_t[0][:], dr["z8"])
        nc.sync.dma_start(xhl_t[1][:], dr["xhl"][1])
        nc.scalar.dma_start(a1_t[1][:], dr["z8"])
        nc.scalar.dma_start(xf_t[0][:], dr["xf"][0])
        nc.sync.dma_start(xhl_t[2][:], dr["xhl"][2])
        nc.scalar.dma_start(xf_t[1][:], dr["xf"][1])
        nc.scalar.dma_start(a1_t[2][:], dr["z8"])
        nc.scalar.dma_start(xf_t[2][:], dr["xf"][2])
        for k in range(3, NB):
            nc.scalar.dma_start(a1_t[k][:], dr["z8"])

        # warm-up: ramp the PE p-state on zero matmuls so the first real
        # conv1 starts closer to full clock
        wz = cpool.tile([C, 20, 32], F32R, tag="wz")
        nc.vector.memset(wz[:].bitcast(F32), 0.0)
        if WARMUP:
            wps = pp1.tile([C, 1024], F32, tag="ps")
            for _ in range(WARMUP):
                nc.tensor.matmul(wps[:, 0:512], wz[:, 0:4, :], wz[:, 4:20, :],
                                 start=True, stop=True)

        def _dr_win(full, pstride, row, kx, nrows=16):
            # (hi,lo) pair window: [part, pair(2), rows, cols]; pair delta is
            # one plane (=WP elements)
            off = row * 2 * WP + kx
            return AP(full.tensor, full.offset + off,
                      [[pstride, C], [WP, 2], [2 * WP, nrows], [1, W]])

        def _conv1(i):
            xhl = xhl_t[i % NB]
            a1 = a1_t[i % NB]
            full = xhl[:]
            pstride = full.ap[0][0]
            ps1 = pp1.tile([C, 1024], F32, tag="ps")
            for h in (0, 1):
                rs = 16 * h
                out_ap = ps1[:, 512 * h:512 * (h + 1)]
                for t, (ky, kx) in enumerate(TAPS):
                    nc.tensor.matmul(out_ap, w1t8[:, 2 * t:2 * t + 2, :],
                                     _dr_win(full, pstride, rs + ky, kx),
                                     start=(t == 0), stop=(t == 6),
                                     perf_mode=MM)
            # stage1: a1 = rne_u8(clip(inv1/15*ps + b1s, 0, 15)) in 2 ops
            rt = spool.tile([C, H, W], F32, tag="st_r", name="rt")
            nc.scalar.activation(rt[:],
                                 ps1[:].rearrange("c (h w) -> c h w", h=H),
                                 AF.Identity, bias=b1s, scale=inv1)
            eng = nc.vector if A1U_ENG == "dve" else nc.gpsimd
            eng.tensor_scalar(a1[:, 1:H + 1, 1:W + 1], rt[:],
                              0.0, 15.0, OP.max, OP.min)
            if DEBUG_A1:
                nc.sync.dma_start(dr["a1d"][i], a1[:, 1:H + 1, 1:W + 1])

        def _conv2(i, per_half=False):
            xhl = xhl_t[i % NB]
            a1 = a1_t[i % NB]
            xf = xf_t[i % NB]
            y8 = opool.tile([C, H, W], U8, tag="y8")
            full = a1[:].bitcast(FP8)
            pstride = full.ap[0][0]
            ps2 = pp2.tile([C, 1024], F32, tag="ps")

            def _mm(h):
                rs = 16 * h
                out_ap = ps2[:, 512 * h:512 * (h + 1)]
                for k in range(4):
                    ky, kx = SLOT_TAPS[2 * k]
                    off = (rs + ky) * WP + kx
                    rhs = AP(full.tensor, full.offset + off,
                             [[pstride, C], [WP, 2], [WP, 16], [1, W]])
                    nc.tensor.matmul(out_ap, w2t8[:, 2 * k:2 * k + 2, :], rhs,
                                     start=(k == 0), stop=(k == 3),
                                     perf_mode=MM)

            def _st2(h0, h1):
                # rows [16*h0, 16*h1): y8 = rne_u8(clip(sc2*ps+b2s+15x, 0, 15))
                rs, re = 16 * h0, 16 * h1
                ps_ap = ps2[:, rs * W:re * W].rearrange(
                    "c (h w) -> c h w", h=re - rs)
                gt = spool.tile([C, H, W], F32, tag="st_g", name="gt")
                g = gt[:, rs:re, :]
                nc.scalar.activation(g, ps_ap, AF.Identity, bias=b2s,
                                     scale=sc2)
                hht = spool.tile([C, H, W], F32, tag="st_h", name="hht")
                hh = hht[:, rs:re, :]
                heng = nc.gpsimd if HH_ENG == "pool" else nc.vector
                heng.tensor_tensor(hh, g, xf[:, rs:re, :], OP.add)
                yeng = nc.vector if Y8_ENG == "dve" else nc.gpsimd
                yeng.tensor_scalar(y8[:, rs:re, :], hh, 0.0, 15.0,
                                   OP.max, OP.min)
                nc.sync.dma_start(dr["y"][i][:, rs:re, :], y8[:, rs:re, :])

            if per_half:
                _mm(0)
                _st2(0, 1)
                _mm(1)
                _st2(1, 2)
            else:
                _mm(0)
                _mm(1)
                _st2(0, 2)

        # distance-2 software pipeline: conv2(i) trails conv1(i) by two
        # iterations so stage1(i) hides behind conv1(i+1)/conv1(i+2) PE work.
        for i in range(bl + DIST):
            nxt = i + PREF
            if i < bl:
                _conv1(i)
                if 2 < nxt < bl:
                    nc.sync.dma_start(xhl_t[nxt % NB][:], dr["xhl"][nxt])
            if i >= DIST:
                _conv2(i - DIST,
                       per_half=(LASTHALF and i == bl + DIST - 1))
            # xf(nxt) lands in the buffer conv2(i-DIST) just read; issue the
            # prefetch after that read so the WAR resolves correctly.
            if 2 < nxt < bl:
                nc.scalar.dma_start(xf_t[nxt % NB][:], dr["xf"][nxt])


def _build(bl=BL):
    nc = bacc.Bacc("TRN2", target_bir_lowering=False, debug=False,
                   enable_asserts=False, num_devices=NCORES)
    dr = {}
    dr["xhl"] = nc.dram_tensor("xhl", [bl, C, HP, 2, WP], FP8,
                               kind="ExternalInput").ap()
    dr["xf"] = nc.dram_tensor("xf", [bl, C, H, W], F16,
                              kind="ExternalInput").ap()
    dr["w1t8"] = nc.dram_tensor("w1t8", [C, 14, C], FP8,
                                kind="ExternalInput").ap()
    dr["w2t8"] = nc.dram_tensor("w2t8", [C, 8, C], FP8,
                                kind="ExternalInput").ap()
    dr["bnp"] = nc.dram_tensor("bnp", [C, 4], F32, kind="ExternalInput").ap()
    dr["z8"] = nc.dram_tensor("z8", [C, (HP + 1) * WP], U8,
                              kind="ExternalInput").ap()
    dr["y"] = nc.dram_tensor("y", [bl, C, H, W], U8, kind="ExternalOutput").ap()
    if DEBUG_A1:
        dr["a1d"] = nc.dram_tensor("a1d", [bl, C, H, W], U8,
                                   kind="ExternalOutput").ap()
    with tile.TileContext(nc) as tc:
        _emit(tc, dr, bl)
    nc.compile()
    return nc


_CACHED = None


def _host_prep(inputs):
    """Replicate the reference's fp32 weight-quant + BN folding in numpy."""
    import ml_dtypes
    f = lambda v: np.asarray(v, dtype=np.float32)

    def wint(w):
        t = np.tanh(f(w))
        m = np.abs(t).max()
        t2 = t / (np.float32(2.0) * m) + np.float32(0.5)
        v = t2 * np.float32(15.0)
        return (np.float32(2.0) * np.round(v) - np.float32(15.0)).astype(np.float32)

    wi1 = wint(inputs["w1"]).reshape(C, C, 3, 3)
    wi2 = wint(inputs["w2"]).reshape(C, C, 3, 3)
    w1t8 = np.empty((C, 14, C), np.float32)
    for t, (ky, kx) in enumerate(TAPS):
        w1t8[:, 2 * t, :] = wi1[:, :, ky, kx].T
        w1t8[:, 2 * t + 1, :] = wi1[:, :, ky, kx].T
    w2t8 = np.zeros((C, 8, C), np.float32)
    for s, st in enumerate(SLOT_TAPS):
        if st is not None:
            w2t8[:, s, :] = wi2[:, :, st[0], st[1]].T

    g1, b1, m1, v1, g2, b2, m2, v2 = (
        f(inputs[k]) for k in ("gamma1", "beta1", "mean1", "var1",
                               "gamma2", "beta2", "mean2", "var2"))
    inv1 = g1 / np.sqrt(v1 + np.float32(EPS))
    inv2 = g2 / np.sqrt(v2 + np.float32(EPS))
    b1s = np.float32(15.0) * (b1 - m1 * inv1)
    b2s = np.float32(15.0) * (b2 - m2 * inv2)
    inv1_15 = inv1 / np.float32(15.0)
    sc2p = np.float32(512.0) * inv2 / np.float32(15.0)
    bnp = np.ascontiguousarray(np.stack([inv1_15, b1s, sc2p, b2s], axis=1))
    return (np.ascontiguousarray(w1t8.astype(ml_dtypes.float8_e4m3fn)),
            np.ascontiguousarray(w2t8.astype(ml_dtypes.float8_e4m3fn)), bnp)


def _split_x(x):
    """Host hi/lo fp8 split of 15x into the padded interleaved layout."""
    import ml_dtypes
    xs = np.float32(15.0) * np.asarray(x, np.float32)  # [n, C, H, W]
    hi = xs.astype(ml_dtypes.float8_e4m3fn)
    lo = (xs - hi.astype(np.float32)).astype(ml_dtypes.float8_e4m3fn)
    n = xs.shape[0]
    xhl = np.zeros((n, C, HP, 2, WP), ml_dtypes.float8_e4m3fn)
    xhl[:, :, 1:H + 1, 0, 1:W + 1] = hi
    xhl[:, :, 1:H + 1, 1, 1:W + 1] = lo
    return np.ascontiguousarray(xhl), np.ascontiguousarray(xs.astype(np.float16))


def _in_maps(inputs, bl=BL, ncores=NCORES):
    w1t8, w2t8, bnp = _host_prep(inputs)
    base = {"w1t8": w1t8, "w2t8": w2t8, "bnp": bnp,
            "z8": np.zeros((C, (HP + 1) * WP), np.uint8)}
    x = np.asarray(inputs["x"], dtype=np.float32)
    maps = []
    for c in range(ncores):
        m = dict(base)
        m["xhl"], m["xf"] = _split_x(x[c * bl:(c + 1) * bl])
        maps.append(m)
    return maps


def _run(inputs, trace=False):
    global _CACHED
    if _CACHED is None:
        _CACHED = _build()
    res = run_bass_kernel_spmd(_CACHED, _in_maps(inputs),
                               core_ids=list(range(NCORES)), trace=trace)
    y = np.concatenate([np.asarray(res.results[c]["y"]).astype(np.float32)
                        for c in range(NCORES)], axis=0)
    return y * np.float32(1.0 / 15.0), res


def kernel(**inputs) -> np.ndarray:
    y, _ = _run(inputs, trace=False)
    return y


# revision 10
# speedup vs baseline: 1.4354x; 1.2623x over previous
"""Trainium2 Bass kernel for quantized BasicBlock (DoReFa conv-bn-quant x2 + skip).

Strategy (75.7us prior -> target ~42us):
- Data-parallel over batch: 128 images -> 16 per core across 8 cores.
- Weights DoReFa-quantized to odd ints in [-15,15] on the HOST (exact fp32
  replication of the reference math); exact in fp8e4.
- x is split on the host: hi = fp8(15x), lo = fp8(15x - hi); sent as a
  padded row-interleaved [C, 34, 2, 34] fp8 tensor. conv1 = 7 fp8 DoubleRow
  matmuls per half (pair = (hi,lo) windows of the same tap, both slots
  carrying the same integer weight) -> K=256, 0.5 cyc/row.
- stage1 uses the HW's round-to-nearest-even f32->uint8 convert: one ACT
  affine (Identity, scale=inv1/15, bias=15(b-mu*inv)) + one DVE dual-op
  tensor_scalar (max 0, min 15) writing uint8 -> a1 holds exact ints 0..15.
- conv2 reads a1 BITCAST as fp8e4: uint8 k in 0..15 bitcasts to the exactly
  linear subnormal/low-normal values k*2^-9, so psum = 2^-9 * int-conv;
  the 2^9 is folded into the stage-2 scale. 4 DR matmuls per half.
- skip: host also sends xf16 = fp16(15x); hh = g + xf16 (one mixed-dtype
  tensor_tensor on Pool), y = RNE-uint8(clip(hh,0,15)) on DVE, decoded /15
  on host.
- schedule: distance-2 software pipeline; 5 rotating input buffers; PE
  warm-up matmuls ramp the p-state during the DMA fill; last image runs
  per-half to shorten the drain tail.
Steady state per image: PE 22 DR matmuls (2395ns), DVE 2394ns, ACT 2190ns,
Pool 2187ns, DMA 2076ns.
"""
import numpy as np

import concourse.bass as bass
import concourse.tile as tile
from concourse import bacc, mybir, masks
from concourse.ap import AP
from concourse.bass_utils import run_bass_kernel_spmd

AF = mybir.ActivationFunctionType
OP = mybir.AluOpType
F32 = mybir.dt.float32
F32R = mybir.dt.float32r
FP8 = mybir.dt.float8e4
F16 = mybir.dt.float16
U8 = mybir.dt.uint8
MM = mybir.MatmulPerfMode.DoubleRow

B, C, H, W = 128, 128, 32, 32
NCORES = 8
BL = B // NCORES          # images per core
HP, WP = H + 2, W + 2     # zero-padded image
EPS = 1e-5
NB = 5                    # rotating input-buffer depth
WARMUP = 6                # dummy PE warm-up matmuls before the main loop
PSB1, PSB2 = 2, 2         # psum pool depths ([C,1024] tiles; 2 banks each)
SPB = 3                   # stage pool depth
OPB = 3                   # out pool depth
DIST = 2                  # conv2 trails conv1 by DIST images
PREF = 3                  # input prefetch distance (images ahead)
LASTHALF = True           # per-half epilogue for the last image
HH_ENG = "dve"            # engine for hh = g + xf16 (DVE 2x: ~594ns)
A1U_ENG = "dve"           # engine for the stage1 uint8 quantize
Y8_ENG = "dve"            # engine for the stage2 uint8 quantize
DEBUG_A1 = False          # add a debug output dumping stage-1 a1 ints

TAPS = [(0, 1), (0, 2), (1, 0), (1, 1), (1, 2), (2, 0), (2, 1)]  # (0,0),(2,2) pruned
# conv2 DoubleRow slot order: pairs with constant +1-row (=WP elements) delta.
SLOT_TAPS = [(0, 1), (1, 1), (0, 2), (1, 2), (1, 0), (2, 0), (2, 1), None]


def _emit(tc, dr, bl):
    nc = tc.nc
    with tc.tile_pool(name="const", bufs=1) as cpool, \
         tc.tile_pool(name="stage", bufs=SPB) as spool, \
         tc.tile_pool(name="out", bufs=OPB) as opool, \
         tc.tile_pool(name="ps1", bufs=PSB1, space="PSUM") as pp1, \
         tc.tile_pool(name="ps2", bufs=PSB2, space="PSUM") as pp2:

        # rotating input buffers: xhl holds the padded (hi,lo) fp8 planes
        # (borders pre-zeroed on the host), a1 gets zero borders via DMA.
        xhl_t = [cpool.tile([C, HP, 2, WP], FP8, tag=f"xhl{k}", name=f"xhl{k}")
                 for k in range(NB)]
        a1_t = [cpool.tile([C, HP + 1, WP], U8, tag=f"a1{k}", name=f"a1{k}")
                for k in range(NB)]
        xf_t = [cpool.tile([C, H, W], F16, tag=f"xf{k}", name=f"xf{k}")
                for k in range(NB)]

        # image 0 lands in two chunks with the weights between them: conv1(0)
        # h0 only needs padded rows 0..17.
        nc.sync.dma_start(xhl_t[0][:, 0:18, :, :], dr["xhl"][0][:, 0:18, :, :])
        w1t8 = cpool.tile([C, 14, C], FP8, tag="w1t8", name="w1t8")
        nc.gpsimd.dma_start(w1t8[:], dr["w1t8"])
        nc.sync.dma_start(xhl_t[0][:, 18:HP, :, :], dr["xhl"][0][:, 18:HP, :, :])
        w2t8 = cpool.tile([C, 8, C], FP8, tag="w2t8", name="w2t8")
        nc.gpsimd.dma_start(w2t8[:], dr["w2t8"])
        # bn affines, host-folded: [inv1/15, b1s, 512*inv2/15, b2s]
        bnp = cpool.tile([C, 4], F32, tag="bnp")
        nc.gpsimd.dma_start(bnp[:], dr["bnp"])
        inv1, b1s, sc2, b2s = (bnp[:, k:k + 1] for k in range(4))

        # a1(0)/a1(1) borders must land before conv2(0)/conv2(1); the rest of
        # the zero fills can trail the early image/skip transfers.
        nc.gpsimd.dma_start(a1_t[0][:], dr["z8"])
        nc.sync.dma_start(xhl_t[1][:], dr["xhl"][1])
        nc.gpsimd.dma_start(a1_t[1][:], dr["z8"])
        nc.gpsimd.dma_start(xf_t[0][:], dr["xf"][0])
        nc.sync.dma_start(xhl_t[2][:], dr["xhl"][2])
        nc.gpsimd.dma_start(xf_t[1][:], dr["xf"][1])
        nc.gpsimd.dma_start(a1_t[2][:], dr["z8"])
        nc.gpsimd.dma_start(xf_t[2][:], dr["xf"][2])
        for k in range(3, NB):
            nc.gpsimd.dma_start(a1_t[k][:], dr["z8"])

        # warm-up: ramp the PE p-state on zero matmuls so the first real
        # conv1 starts closer to full clock
        wz = cpool.tile([C, 20, 32], F32R, tag="wz")
        nc.vector.memset(wz[:].bitcast(F32), 0.0)
        if WARMUP:
            wps = pp1.tile([C, 1024], F32, tag="ps")
            for _ in range(WARMUP):
                nc.tensor.matmul(wps[:, 0:512], wz[:, 0:4, :], wz[:, 4:20, :],
                                 start=True, stop=True)

        def _dr_win(full, pstride, row, kx, nrows=16):
            # (hi,lo) pair window: [part, pair(2), rows, cols]; pair delta is
            # one plane (=WP elements)
            off = row * 2 * WP + kx
            return AP(full.tensor, full.offset + off,
                      [[pstride, C], [WP, 2], [2 * WP, nrows], [1, W]])

        def _conv1(i):
            xhl = xhl_t[i % NB]
            a1 = a1_t[i % NB]
            full = xhl[:]
            pstride = full.ap[0][0]
            ps1 = pp1.tile([C, 1024], F32, tag="ps")
            for h in (0, 1):
                rs = 16 * h
                out_ap = ps1[:, 512 * h:512 * (h + 1)]
                for t, (ky, kx) in enumerate(TAPS):
                    nc.tensor.matmul(out_ap, w1t8[:, 2 * t:2 * t + 2, :],
                                     _dr_win(full, pstride, rs + ky, kx),
                                     start=(t == 0), stop=(t == 6),
                                     perf_mode=MM)
            # stage1: a1 = rne_u8(clip(inv1/15*ps + b1s, 0, 15)) in 2 ops
            rt = spool.tile([C, H, W], F32, tag="st_r", name="rt")
            nc.scalar.activation(rt[:],
                                 ps1[:].rearrange("c (h w) -> c h w", h=H),
                                 AF.Identity, bias=b1s, scale=inv1)
            eng = nc.vector if A1U_ENG == "dve" else nc.gpsimd
            eng.tensor_scalar(a1[:, 1:H + 1, 1:W + 1], rt[:],
                              0.0, 15.0, OP.max, OP.min)
            if DEBUG_A1:
                nc.sync.dma_start(dr["a1d"][i], a1[:, 1:H + 1, 1:W + 1])

        def _conv2(i, per_half=False):
            xhl = xhl_t[i % NB]
            a1 = a1_t[i % NB]
            xf = xf_t[i % NB]
            y8 = opool.tile([C, H, W], U8, tag="y8")
            full = a1[:].bitcast(FP8)
            pstride = full.ap[0][0]
            ps2 = pp2.tile([C, 1024], F32, tag="ps")

            def _mm(h):
                rs = 16 * h
                out_ap = ps2[:, 512 * h:512 * (h + 1)]
                for k in range(4):
                    ky, kx = SLOT_TAPS[2 * k]
                    off = (rs + ky) * WP + kx
                    rhs = AP(full.tensor, full.offset + off,
                             [[pstride, C], [WP, 2], [WP, 16], [1, W]])
                    nc.tensor.matmul(out_ap, w2t8[:, 2 * k:2 * k + 2, :], rhs,
                                     start=(k == 0), stop=(k == 3),
                                     perf_mode=MM)

            def _st2(h0, h1):
                # rows [16*h0, 16*h1): y8 = rne_u8(clip(sc2*ps+b2s+15x, 0, 15))
                rs, re = 16 * h0, 16 * h1
                ps_ap = ps2[:, rs * W:re * W].rearrange(
                    "c (h w) -> c h w", h=re - rs)
                gt = spool.tile([C, H, W], F32, tag="st_g", name="gt")
                g = gt[:, rs:re, :]
                nc.scalar.activation(g, ps_ap, AF.Identity, bias=b2s,
                                     scale=sc2)
                hht = spool.tile([C, H, W], F32, tag="st_h", name="hht")
                hh = hht[:, rs:re, :]
                heng = nc.gpsimd if HH_ENG == "pool" else nc.vector
                heng.tensor_tensor(hh, g, xf[:, rs:re, :], OP.add)
                yeng = nc.vector if Y8_ENG == "dve" else nc.gpsimd
                yeng.tensor_scalar(y8[:, rs:re, :], hh, 0.0, 15.0,
                                   OP.max, OP.min)
                nc.sync.dma_start(dr["y"][i][:, rs:re, :], y8[:, rs:re, :])

            if per_half:
                _mm(0)
                _st2(0, 1)
                _mm(1)
                _st2(1, 2)
            else:
                _mm(0)
                _mm(1)
                _st2(0, 2)

        # distance-2 software pipeline: conv2(i) trails conv1(i) by two
        # iterations so stage1(i) hides behind conv1(i+1)/conv1(i+2) PE work.
        for i in range(bl + DIST):
            nxt = i + PREF
            if i < bl:
                _conv1(i)
                if 2 < nxt < bl:
                    nc.sync.dma_start(xhl_t[nxt % NB][:], dr["xhl"][nxt])
            if i >= DIST:
                _conv2(i - DIST,
                       per_half=(LASTHALF and i == bl + DIST - 1))
            # xf(nxt) lands in the buffer conv2(i-DIST) just read; issue the
            # prefetch after that read so the WAR resolves correctly.
            if 2 < nxt < bl:
                nc.gpsimd.dma_start(xf_t[nxt % NB][:], dr["xf"][nxt])


def _build(bl=BL):
    nc = bacc.Bacc("TRN2", target_bir_lowering=False, debug=False,
                   enable_asserts=False, num_devices=NCORES)
    dr = {}
    dr["xhl"] = nc.dram_tensor("xhl", [bl, C, HP, 2, WP], FP8,
                               kind="ExternalInput").ap()
    dr["xf"] = nc.dram_tensor("xf", [bl, C, H, W], F16,
                              kind="ExternalInput").ap()
    dr["w1t8"] = nc.dram_tensor("w1t8", [C, 14, C], FP8,
                                kind="ExternalInput").ap()
    dr["w2t8"] = nc.dram_tensor("w2t8", [C, 8, C], FP8,
                                kind="ExternalInput").ap()
    dr["bnp"] = nc.dram_tensor("bnp", [C, 4], F32, kind="ExternalInput").ap()
    dr["z8"] = nc.dram_tensor("z8", [C, (HP + 1) * WP], U8,
                              kind="ExternalInput").ap()
    dr["y"] = nc.dram_tensor("y", [bl, C, H, W], U8, kind="ExternalOutput").ap()
    if DEBUG_A1:
        dr["a1d"] = nc.dram_tensor("a1d", [bl, C, H, W], U8,
                                   kind="ExternalOutput").ap()
    with tile.TileContext(nc) as tc:
        _emit(tc, dr, bl)
    nc.compile()
    return nc


_CACHED = None


def _host_prep(inputs):
    """Replicate the reference's fp32 weight-quant + BN folding in numpy."""
    import ml_dtypes
    f = lambda v: np.asarray(v, dtype=np.float32)

    def wint(w):
        t = np.tanh(f(w))
        m = np.abs(t).max()
        t2 = t / (np.float32(2.0) * m) + np.float32(0.5)
        v = t2 * np.float32(15.0)
        return (np.float32(2.0) * np.round(v) - np.float32(15.0)).astype(np.float32)

    wi1 = wint(inputs["w1"]).reshape(C, C, 3, 3)
    wi2 = wint(inputs["w2"]).reshape(C, C, 3, 3)
    w1t8 = np.empty((C, 14, C), np.float32)
    for t, (ky, kx) in enumerate(TAPS):
        w1t8[:, 2 * t, :] = wi1[:, :, ky, kx].T
        w1t8[:, 2 * t + 1, :] = wi1[:, :, ky, kx].T
    w2t8 = np.zeros((C, 8, C), np.float32)
    for s, st in enumerate(SLOT_TAPS):
        if st is not None:
            w2t8[:, s, :] = wi2[:, :, st[0], st[1]].T

    g1, b1, m1, v1, g2, b2, m2, v2 = (
        f(inputs[k]) for k in ("gamma1", "beta1", "mean1", "var1",
                               "gamma2", "beta2", "mean2", "var2"))
    inv1 = g1 / np.sqrt(v1 + np.float32(EPS))
    inv2 = g2 / np.sqrt(v2 + np.float32(EPS))
    b1s = np.float32(15.0) * (b1 - m1 * inv1)
    b2s = np.float32(15.0) * (b2 - m2 * inv2)
    inv1_15 = inv1 / np.float32(15.0)
    sc2p = np.float32(512.0) * inv2 / np.float32(15.0)
    bnp = np.ascontiguousarray(np.stack([inv1_15, b1s, sc2p, b2s], axis=1))
    return (np.ascontiguousarray(w1t8.astype(ml_dtypes.float8_e4m3fn)),
            np.ascontiguousarray(w2t8.astype(ml_dtypes.float8_e4m3fn)), bnp)


def _split_x(x):
    """Host hi/lo fp8 split of 15x into the padded interleaved layout."""
    import ml_dtypes
    xs = np.float32(15.0) * np.asarray(x, np.float32)  # [n, C, H, W]
    hi = xs.astype(ml_dtypes.float8_e4m3fn)
    lo = (xs - hi.astype(np.float32)).astype(ml_dtypes.float8_e4m3fn)
    n = xs.shape[0]
    xhl = np.zeros((n, C, HP, 2, WP), ml_dtypes.float8_e4m3fn)
    xhl[:, :, 1:H + 1, 0, 1:W + 1] = hi
    xhl[:, :, 1:H + 1, 1, 1:W + 1] = lo
    return np.ascontiguousarray(xhl), np.ascontiguousarray(xs.astype(np.float16))


def _in_maps(inputs, bl=BL, ncores=NCORES):
    w1t8, w2t8, bnp = _host_prep(inputs)
    base = {"w1t8": w1t8, "w2t8": w2t8, "bnp": bnp,
            "z8": np.zeros((C, (HP + 1) * WP), np.uint8)}
    x = np.asarray(inputs["x"], dtype=np.float32)
    maps = []
    for c in range(ncores):
        m = dict(base)
        m["xhl"], m["xf"] = _split_x(x[c * bl:(c + 1) * bl])
        maps.append(m)
    return maps


def _run(inputs, trace=False):
    global _CACHED
    if _CACHED is None:
        _CACHED = _build()
    res = run_bass_kernel_spmd(_CACHED, _in_maps(inputs),
                               core_ids=list(range(NCORES)), trace=trace)
    y = np.concatenate([np.asarray(res.results[c]["y"]).astype(np.float32)
                        for c in range(NCORES)], axis=0)
    return y * np.float32(1.0 / 15.0), res


def kernel(**inputs) -> np.ndarray:
    y, _ = _run(inputs, trace=False)
    return y
